# revision 10
# baseline (speedup 1.0000x reference)
"""Trainium2 Bass kernel for nn_Dependency: 2-layer bi-LSTM single step over
S=4096 tokens + upper-triangular pairwise score matrix.

Sharding: 8 cores; core c owns tokens [512c, 512c+512) for the LSTM.
The (symmetric) S x S score matrix's upper triangle is computed as one
512x512 diagonal block per core (statically masked) plus 7 of the 56
off-diagonal 512x256 half-blocks per core, assigned via a per-core job
table whose offsets feed dynamic-DRAM-offset DMAs. Full `out` is shared
across cores with one AllGather of the feature-major outT [200, 512].
"""
import os

import numpy as np

import concourse.bass as bass
import concourse.mybir as mybir
import concourse.tile as tile
from concourse import bacc
from concourse.bass import ds
from concourse.bass_utils import run_bass_kernel_spmd

NC = 8
S = 4096
SL = S // NC          # tokens per core
H = 100
H2 = 2 * H
IN0 = 768             # l0 input features = 300+45+23+200+200
K0 = IN0 // 128       # 6 k-tiles of 128 for layer-0 Wih
NJOB = 7              # off-diagonal jobs per core (56 half-blocks / 8)

_MM_DT_NAME = os.environ.get("BASS_MM_DT", "float32r")
_STAGE = int(os.environ.get("BASS_STAGE", "4"))
F32 = mybir.dt.float32

_CACHE = {}


def _mmdt():
    return getattr(mybir.dt, _MM_DT_NAME)


def _build():
    """Trace + compile the single SPMD program. Cached per process."""
    key = (_MM_DT_NAME, _STAGE)
    if key in _CACHE:
        return _CACHE[key]

    MDT = _mmdt()

    nc = bacc.Bacc("TRN2", target_bir_lowering=False, debug=False,
                   num_devices=NC)

    # ---- I/O ----
    gt_d = nc.dram_tensor("gt", [128, K0 * SL], MDT, kind="ExternalInput")
    w0f_d = nc.dram_tensor("w0f", [128, K0 * 400], MDT, kind="ExternalInput")
    w0b_d = nc.dram_tensor("w0b", [128, K0 * 400], MDT, kind="ExternalInput")
    w1f_d = nc.dram_tensor("w1f", [100, 2 * 400], MDT, kind="ExternalInput")
    w1b_d = nc.dram_tensor("w1b", [100, 2 * 400], MDT, kind="ExternalInput")
    whh_d = nc.dram_tensor("whh", [100, 4 * 400], MDT, kind="ExternalInput")
    bias_d = nc.dram_tensor("bias", [100, 16], F32, kind="ExternalInput")
    h0t_d = nc.dram_tensor("h0t", [100, 4 * SL], MDT, kind="ExternalInput")
    c0t_d = nc.dram_tensor("c0t", [100, 4 * SL], F32, kind="ExternalInput")
    wv_d = nc.dram_tensor("wv", [100, 2], F32, kind="ExternalInput")
    fcb_d = nc.dram_tensor("fcb", [128, 1], F32, kind="ExternalInput")
    jt_d = nc.dram_tensor("jt", [NJOB, 4], mybir.dt.uint32, kind="ExternalInput")

    diag_d = nc.dram_tensor("diag", [512, 512], F32, kind="ExternalOutput")
    offd_d = nc.dram_tensor("offd", [NJOB * 512, 256], F32, kind="ExternalOutput")
    outp_d = nc.dram_tensor("outp", [H2, SL], MDT, kind="ExternalOutput")

    with tile.TileContext(nc) as tc:
        with (
            tc.tile_pool(name="const", bufs=1) as constp,
            tc.tile_pool(name="work", bufs=3) as workp,
            tc.tile_pool(name="gates", bufs=10) as gatep,
            tc.tile_pool(name="jobs", bufs=3) as jobp,
            tc.tile_pool(name="evict", bufs=6) as evictp,
            tc.tile_pool(name="ps", bufs=8, space="PSUM") as psp,
            tc.tile_pool(name="dram", bufs=1, space="DRAM") as dramp,
        ):
            # ---- load constants / inputs into SBUF ----
            def load(pool, dram_t, shape, dtype=F32, eng=None):
                t = pool.tile(shape, dtype, tag=f"c_{dram_t.name}")
                (eng or nc.gpsimd).dma_start(t[:], dram_t[:])
                return t

            gt_sb = load(constp, gt_d, [128, K0 * SL], MDT)
            w0_sb = {"f": load(constp, w0f_d, [128, K0 * 400], MDT),
                     "b": load(constp, w0b_d, [128, K0 * 400], MDT)}
            w1_sb = {"f": load(constp, w1f_d, [100, 800], MDT),
                     "b": load(constp, w1b_d, [100, 800], MDT)}
            whh_sb = load(constp, whh_d, [100, 1600], MDT)
            bias_sb = load(constp, bias_d, [100, 16])
            h0t_sb = load(constp, h0t_d, [100, 4 * SL], MDT)
            c0t_sb = load(constp, c0t_d, [100, 4 * SL])
            wv_sb = load(constp, wv_d, [100, 2])
            fcb_sb = load(constp, fcb_d, [128, 1])
            jt_sb = load(constp, jt_d, [NJOB, 4], mybir.dt.uint32, eng=nc.sync)

            # triangular mask Mbig[x, u] = 1.0 if u >= x + 384 else 0.0
            mbig = constp.tile([128, 896], F32, tag="c_mbig")
            nc.gpsimd.memset(mbig[:], 1.0)
            nc.gpsimd.affine_select(
                out=mbig[:], in_=mbig[:], pattern=[[1, 896]],
                channel_multiplier=-1, base=-384,
                compare_op=mybir.AluOpType.is_ge, fill=0.0)

            ACT = mybir.ActivationFunctionType

            # ---- one LSTM cell step; returns h [100, SL] in SBUF ----
            def cell(cell_idx, rhs_list, bias_col0):
                # rhs_list: list of (lhsT_tile, col_offset, ktile_rows, rhs_ap)
                sig = {}
                tanh_g = None
                ps_gates = []
                for g in range(4):
                    ps = psp.tile([100, SL], F32, tag="ps")
                    n_in = len(rhs_list)
                    for ki, (wt, coff, krows, rhs_ap) in enumerate(rhs_list):
                        nc.tensor.matmul(
                            ps[:], wt[0:krows, coff + g * 100:coff + (g + 1) * 100],
                            rhs_ap, start=(ki == 0), stop=(ki == n_in - 1))
                    ps_gates.append(ps)
                # activations: i, f, o -> sigmoid; g -> tanh (bias fused)
                for g, ps in enumerate(ps_gates):
                    bap = bias_sb[:, bias_col0 + g:bias_col0 + g + 1]
                    t = gatep.tile([100, SL], F32, tag="gt")
                    func = ACT.Tanh if g == 2 else ACT.Sigmoid
                    nc.scalar.activation(t[:], ps[:], func, bias=bap)
                    if g == 2:
                        tanh_g = t
                    else:
                        sig[g] = t
                # c2 = sig_f * c0 + sig_i * tanh_g
                t1 = gatep.tile([100, SL], F32, tag="gt")
                nc.vector.tensor_mul(t1[:], sig[0][:], tanh_g[:])
                t2 = gatep.tile([100, SL], F32, tag="gt")
                c0ap = c0t_sb[:, cell_idx * SL:(cell_idx + 1) * SL]
                nc.vector.tensor_mul(t2[:], sig[1][:], c0ap)
                c2 = gatep.tile([100, SL], F32, tag="gt")
                nc.vector.tensor_add(c2[:], t1[:], t2[:])
                tc2 = gatep.tile([100, SL], F32, tag="gt")
                nc.scalar.activation(tc2[:], c2[:], ACT.Tanh)
                h = gatep.tile([100, SL], MDT, tag="h")
                nc.vector.tensor_mul(h[:], sig[3][:], tc2[:])
                return h

            # ---- layer 0 ----
            h_l0 = {}
            for ci, d in enumerate("fb"):
                rhs = [(w0_sb[d], k * 400, 128,
                        gt_sb[:, k * SL:(k + 1) * SL]) for k in range(K0)]
                rhs.append((whh_sb, ci * 400, 100,
                            h0t_sb[:, ci * SL:(ci + 1) * SL]))
                h_l0[d] = cell(ci, rhs, ci * 4)

            # ---- layer 1 ----
            h_l1 = {}
            for i, d in enumerate("fb"):
                ci = 2 + i
                rhs = [(w1_sb[d], 0, 100, h_l0["f"][:]),
                       (w1_sb[d], 400, 100, h_l0["b"][:]),
                       (whh_sb, ci * 400, 100,
                        h0t_sb[:, ci * SL:(ci + 1) * SL])]
                h_l1[d] = cell(ci, rhs, ci * 4)

            hf1, hb1 = h_l1["f"], h_l1["b"]

            # ---- write outT local part; AllGather outT across cores ----
            nc.sync.dma_start(outp_d[0:100, :], hf1[:])
            nc.sync.dma_start(outp_d[100:200, :], hb1[:])

            if _STAGE < 2:
                _noag = True
            ag_in = dramp.tile([H2, SL], MDT)
            ag_out = dramp.tile([NC * H2, SL], MDT, addr_space="Shared")
            if _STAGE >= 3:
                nc.gpsimd.dma_start(ag_in[0:100, :], hf1[:])
                nc.gpsimd.dma_start(ag_in[100:200, :], hb1[:])
                nc.gpsimd.collective_compute(
                    "AllGather", mybir.AluOpType.bypass,
                    replica_groups=[list(range(NC))],
                    ins=[ag_in.opt()], outs=[ag_out.opt()])

            # ---- diagonal 512x512 block (local data only; overlaps AG) ----
            if _STAGE >= 2:
                a0 = workp.tile([100, SL], MDT, tag="a0")
                a1 = workp.tile([100, SL], MDT, tag="a1")
                nc.vector.tensor_scalar_mul(a0[:], hf1[:], wv_sb[:, 0:1])
                nc.vector.tensor_scalar_mul(a1[:], hb1[:], wv_sb[:, 1:2])
            for rt in range(4 if _STAGE >= 2 else 0):
                ps = psp.tile([128, 512], F32, tag="ps")
                nc.tensor.matmul(ps[:], a0[:, rt * 128:(rt + 1) * 128],
                                 hf1[:], start=True, stop=False)
                nc.tensor.matmul(ps[:], a1[:, rt * 128:(rt + 1) * 128],
                                 hb1[:], start=False, stop=True)
                ev = evictp.tile([128, 512], F32, tag="ev")
                # (psum + fc_b) * mask
                nc.vector.scalar_tensor_tensor(
                    out=ev[:], in0=ps[:], scalar=fcb_sb[:, 0:1],
                    in1=mbig[:, 384 - 128 * rt:896 - 128 * rt],
                    op0=mybir.AluOpType.add, op1=mybir.AluOpType.mult)
                nc.sync.dma_start(diag_d[rt * 128:(rt + 1) * 128, :], ev[:])

            # ---- 7 off-diagonal jobs from the job table ----
            for j in range(NJOB if _STAGE >= 4 else 0):
                lhs_r = nc.sync.value_load(jt_sb[j:j + 1, 0:1])
                rhs_r = nc.sync.value_load(jt_sb[j:j + 1, 1:2])
                rhs_c = nc.sync.value_load(jt_sb[j:j + 1, 2:3])

                lh0 = jobp.tile([100, SL], MDT, tag="lh0")
                lh1 = jobp.tile([100, SL], MDT, tag="lh1")
                nc.sync.dma_start(lh0[:], ag_out[ds(lhs_r, 100), :])
                nc.sync.dma_start(lh1[:], ag_out[ds(lhs_r + 100, 100), :])
                rh0 = jobp.tile([100, 256], MDT, tag="rh0")
                rh1 = jobp.tile([100, 256], MDT, tag="rh1")
                nc.sync.dma_start(rh0[:], ag_out[ds(rhs_r, 100), ds(rhs_c, 256)])
                nc.sync.dma_start(rh1[:], ag_out[ds(rhs_r + 100, 100),
                                                 ds(rhs_c, 256)])
                la0 = jobp.tile([100, SL], MDT, tag="la0")
                la1 = jobp.tile([100, SL], MDT, tag="la1")
                nc.vector.tensor_scalar_mul(la0[:], lh0[:], wv_sb[:, 0:1])
                nc.vector.tensor_scalar_mul(la1[:], lh1[:], wv_sb[:, 1:2])
                for rt in range(4):
                    ps = psp.tile([128, 256], F32, tag="ps")
                    nc.tensor.matmul(ps[:], la0[:, rt * 128:(rt + 1) * 128],
                                     rh0[:], start=True, stop=False)
                    nc.tensor.matmul(ps[:], la1[:, rt * 128:(rt + 1) * 128],
                                     rh1[:], start=False, stop=True)
                    ev = evictp.tile([128, 256], F32, tag="ev2")
                    nc.vector.tensor_scalar_add(ev[:], ps[:], fcb_sb[:, 0:1])
                    nc.sync.dma_start(
                        offd_d[j * 512 + rt * 128:j * 512 + (rt + 1) * 128, :],
                        ev[:])

    nc.compile()
    _CACHE[key] = nc
    return nc


def _jobs_for_core(c):
    """7 (I, J, colhalf) off-diagonal upper-triangle half-blocks for core c."""
    alljobs = [(i, j, ch) for i in range(NC) for j in range(i + 1, NC)
               for ch in range(2)]
    return alljobs[c::NC]


def _prepare(x, tags, hn_tags, chunks, hn_chunks, h0, c0,
             Wih_l0f, Whh_l0f, bih_l0f, bhh_l0f,
             Wih_l0b, Whh_l0b, bih_l0b, bhh_l0b,
             Wih_l1f, Whh_l1f, bih_l1f, bhh_l1f,
             Wih_l1b, Whh_l1b, bih_l1b, bhh_l1b,
             wd, fc_w, fc_b):
    f32 = np.float32

    def tile_rows(a, p):
        # [k*p, m] -> [p, k*m] grouping k-tiles of p rows along columns
        k = a.shape[0] // p
        return np.ascontiguousarray(
            a.reshape(k, p, a.shape[1]).transpose(1, 0, 2).reshape(p, -1))

    gt = np.concatenate([hn_chunks, hn_tags, x, tags, chunks], axis=2)[0]
    gtT = np.ascontiguousarray(gt.T.astype(f32))          # [768, S]

    w0fT = tile_rows(np.ascontiguousarray(Wih_l0f.T), 128)  # [128, 2400]
    w0bT = tile_rows(np.ascontiguousarray(Wih_l0b.T), 128)
    w1fT = tile_rows(np.ascontiguousarray(Wih_l1f.T), 100)  # [100, 800]
    w1bT = tile_rows(np.ascontiguousarray(Wih_l1b.T), 100)
    whhT = np.concatenate(
        [np.ascontiguousarray(W.T)
         for W in (Whh_l0f, Whh_l0b, Whh_l1f, Whh_l1b)], axis=1)  # [100, 1600]
    bias = np.concatenate(
        [(bi + bh).reshape(4, 100).T
         for bi, bh in ((bih_l0f, bhh_l0f), (bih_l0b, bhh_l0b),
                        (bih_l1f, bhh_l1f), (bih_l1b, bhh_l1b))],
        axis=1).astype(f32)                                # [100, 16]
    wv = np.ascontiguousarray((wd * fc_w[0]).reshape(2, 100).T.astype(f32))
    fcb = np.full((128, 1), float(fc_b[0]), f32)

    shared = dict(w0f=w0fT, w0b=w0bT, w1f=w1fT, w1b=w1bT, whh=whhT,
                  bias=bias, wv=wv, fcb=fcb)

    in_maps = []
    jobs_per_core = []
    for c in range(NC):
        sl = slice(c * SL, (c + 1) * SL)
        gt_c = tile_rows(np.ascontiguousarray(gtT[:, sl]), 128)  # [128, 3072]
        h0t = np.concatenate(
            [np.ascontiguousarray(h0[k, sl].T) for k in range(4)],
            axis=1).astype(f32)                             # [100, 2048]
        c0t = np.concatenate(
            [np.ascontiguousarray(c0[k, sl].T) for k in range(4)],
            axis=1).astype(f32)
        jobs = _jobs_for_core(c)
        jobs_per_core.append(jobs)
        jt = np.zeros((NJOB, 4), np.uint32)
        for ji, (I, J, ch) in enumerate(jobs):
            jt[ji] = (I * H2, J * H2, ch * 256, 0)
        m = dict(shared)
        m.update(gt=gt_c, h0t=h0t, c0t=c0t, jt=jt)
        in_maps.append(m)
    return in_maps, jobs_per_core


def _assemble(results, jobs_per_core):
    f32 = np.float32
    scores = np.zeros((S, S), f32)
    out = np.empty((1, S, H2), f32)
    for c in range(NC):
        r = results[c]
        out[0, c * SL:(c + 1) * SL, :] = r["outp"].T
        scores[c * SL:(c + 1) * SL, c * SL:(c + 1) * SL] = r["diag"]
        for ji, (I, J, ch) in enumerate(jobs_per_core[c]):
            scores[I * 512:(I + 1) * 512,
                   J * 512 + ch * 256:J * 512 + (ch + 1) * 256] = \
                r["offd"][ji * 512:(ji + 1) * 512, :]
    return scores, out


def kernel(**inputs):
    nc = _build()
    in_maps, jobs_per_core = _prepare(**inputs)
    res = run_bass_kernel_spmd(nc, in_maps, core_ids=list(range(NC)))
    return _assemble(res.results, jobs_per_core)
